# revision 23
# baseline (speedup 1.0000x reference)
"""GATv2Conv batched-graph kernel for Trainium2 (8 NeuronCores, data-parallel).

Problem: B=16384 independent 9-node graphs, C_in=C_out=256, fixed edge list
(16 directed tree edges + 9 self-loops = 25 edges), GATv2 attention.

V2 design (vs the 342us baseline):
  - Cross-block software pipelining: the PE stream for block b interleaves
    block b's projections (always-ready work) with block b-1's score
    matmuls (gated on DVE-made leaky tiles).  This keeps the tensor engine
    continuously busy so it ramps to its full 2.4 GHz p-state (a cold/idle
    PE runs at 1.2 GHz).
  - Edges are ordered self-loops-first then grouped by source node, so the
    per-edge adds+leaky-relu collapse into a few wide strided ops
    (self-loops: one [128, 9*G] op; per-src groups: dst slices with a
    uniform node stride).  Leaky is applied in place on the h tiles.
  - The leaky tiles (score path only) are stored as fp8-e4m3: the score is
    a 256-term dot product, so the elementwise quantization error averages
    out (~0.4% on scores, well inside the 2e-2 budget).
  - True leaky-relu in one op per tile (ACT Prelu / DVE-Pool STT
    max(0.2x, x)); the old separate 0.2*x linear score path (nu/vsel) is
    gone.
  - Aggregation in graph-major via fused scalar_tensor_tensor chains
    (alpha is a per-partition scalar there), fed by a second graph-major
    xl projection (bf16 - it feeds the output directly).
  - bias is handled host-side (zeros in this problem).
"""

import sys

if "/opt/trn_rl_repo" not in sys.path:
    sys.path.insert(0, "/opt/trn_rl_repo")

import numpy as np
import ml_dtypes

import concourse.bass as bass
import concourse.bacc as bacc
import concourse.mybir as mybir
from concourse import tile
from concourse.bass_utils import run_bass_kernel_spmd

F32 = mybir.dt.float32
BF16 = mybir.dt.bfloat16
FP8 = mybir.dt.bfloat16  # bisect: bf16

N_CORES = 8
B_TOTAL = 16384
NEG_SLOPE = 0.2
BC = B_TOTAL // N_CORES          # graphs per core
NN = 9                           # nodes per graph
C = 256                          # channels
G = 256                          # graphs per block
NBLK = BC // G                   # blocks per core
NT = G // 128                    # 128-graph subtiles per block
NGT = NN * G                     # columns per (chunk, block)

# ---- static edge list ----
# Order: 9 self-loops first (edge e = node e), then tree edges grouped by
# SOURCE node, with the dst list of each group an arithmetic sequence so a
# single strided AP covers the whole group.
_ADJ = {0: [1, 3, 5, 7], 1: [0, 2], 2: [1], 3: [0, 4], 4: [3],
        5: [0, 6], 6: [5], 7: [0, 8], 8: [7]}
EDGES = [(d, d) for d in range(NN)]
SRC_GROUPS = []     # (src, [dsts], edge_base)
for _s in range(NN):
    SRC_GROUPS.append((_s, _ADJ[_s], len(EDGES)))
    for _d in _ADJ[_s]:
        EDGES.append((_s, _d))
NE = len(EDGES)     # 25
assert NE == 25
# in-edges per destination (edge indices into EDGES), self-loop first
IN_EDGES = [[e for e, (_s, d) in enumerate(EDGES) if d == dd and _s == dd] +
            [e for e, (_s, d) in enumerate(EDGES) if d == dd and _s != dd]
            for dd in range(NN)]


class Cfg:
    # engine assignment cyclers (per op class)
    add_engines = ("gpsimd", "vector")
    leaky_engines = ("scalar", "vector")
    agg_engines = ("vector",)                # fused STT agg (AP scalar)
    aggi_engines = ("scalar",)               # agg init: copy with scale
    pcopy_engines = ("scalar", "vector")   # proj psum->sbuf copies (no PSUM on gpsimd)
    gcopy_engines = ("scalar", "vector")     # gm psum->sbuf copies
    # prev-block PE closures drained after each proj / gm psum unit
    fill_proj = (0, 0, 2, 2, 2, 2, 3, 3, 3, 3,
                 3, 3, 3, 3, 3, 3, 3, 3, 2, 2)
    fill_gm = 2


def build_program(cfg: Cfg):
    nc = bacc.Bacc("TRN2", target_bir_lowering=False, debug=False)

    def eng(name):
        return {"vector": nc.vector, "gpsimd": nc.gpsimd,
                "scalar": nc.scalar}[name]

    def copy_op(ename, dst_ap, src_ap):
        if ename == "scalar":
            nc.scalar.copy(dst_ap, src_ap)
        else:
            eng(ename).tensor_copy(dst_ap, src_ap)

    def leaky_op(ename, dst_ap, src_ap):
        if ename == "scalar":
            nc.scalar.activation(dst_ap, src_ap,
                                 mybir.ActivationFunctionType.Prelu,
                                 alpha=NEG_SLOPE)
        else:
            eng(ename).scalar_tensor_tensor(
                dst_ap, src_ap, NEG_SLOPE, src_ap,
                op0=mybir.AluOpType.mult, op1=mybir.AluOpType.max)

    # DRAM tensors
    xT_d = nc.dram_tensor("xT", [C, NBLK * NGT], BF16, kind="ExternalInput")
    wl_d = nc.dram_tensor("wl", [C, C], BF16, kind="ExternalInput")
    wr_d = nc.dram_tensor("wr", [C, C], BF16, kind="ExternalInput")
    attbl_d = nc.dram_tensor("attbl", [128, 2 * NE * NE], FP8,
                             kind="ExternalInput")
    smat_d = nc.dram_tensor("smat", [NE, NE], F32, kind="ExternalInput")
    identf_d = nc.dram_tensor("identf", [128, 128], F32, kind="ExternalInput")
    out_d = nc.dram_tensor("out", [BC, NN * C], BF16, kind="ExternalOutput")
    dbg_ex_d = nc.dram_tensor("dbg_ex", [NE, G], F32, kind="ExternalOutput")
    dbg_al_d = nc.dram_tensor("dbg_al", [128, NT * NE], F32,
                              kind="ExternalOutput")
    dbg_st_d = nc.dram_tensor("dbg_st", [128, NE, G], F32,
                              kind="ExternalOutput")

    with tile.TileContext(nc) as tc:
        with (
            tc.tile_pool(name="const", bufs=1) as cpool,
            tc.tile_pool(name="xin", bufs=2) as xpool,
            tc.tile_pool(name="proj", bufs=2) as prpool,
            tc.tile_pool(name="edge", bufs=2) as epool,
            tc.tile_pool(name="soft", bufs=2) as spool,
            tc.tile_pool(name="gm", bufs=2) as gmpool,
            tc.tile_pool(name="outp", bufs=4) as opool,
            tc.tile_pool(name="ps_proj", bufs=2, space="PSUM") as ps_proj,
            tc.tile_pool(name="ps_gm", bufs=2, space="PSUM") as ps_gm,
            tc.tile_pool(name="ps_sc", bufs=2, space="PSUM") as ps_sc,
        ):
            # ---- constants ----
            wl_sb = cpool.tile([128, 2 * C], BF16, tag="wl")
            wr_sb = cpool.tile([128, 2 * C], BF16, tag="wr")
            nc.sync.dma_start(wl_sb[:, 0:C], wl_d[0:128, :])
            nc.sync.dma_start(wl_sb[:, C:2 * C], wl_d[128:256, :])
            nc.sync.dma_start(wr_sb[:, 0:C], wr_d[0:128, :])
            nc.sync.dma_start(wr_sb[:, C:2 * C], wr_d[128:256, :])
            attbl_sb = cpool.tile([128, 2 * NE * NE], FP8, tag="attbl")
            nc.sync.dma_start(attbl_sb[:], attbl_d[:])
            smat_sb = cpool.tile([NE, NE], F32, tag="smat")
            nc.sync.dma_start(smat_sb[:], smat_d[:])
            identf_sb = cpool.tile([128, 128], F32, tag="identf")
            nc.sync.dma_start(identf_sb[:], identf_d[:])

            ecyc = {"add": 0, "leaky": 0, "agg": 0, "aggi": 0,
                    "pcopy": 0, "gcopy": 0}

            def cyc(kind):
                lst = getattr(cfg, kind + "_engines")
                e = lst[ecyc[kind] % len(lst)]
                ecyc[kind] += 1
                return e

            def emit_edge_ops(pv):
                """DVE-class ops producing this block's leaky tiles (st,
                fp8).  The adds write st directly; leaky is applied in
                place.  Consumed by next iteration's score matmuls."""
                xlT, xrT = pv["xlT"], pv["xrT"]
                st = pv["st"]
                for dch in range(2):
                    for e, (s, d) in enumerate(EDGES):
                        h = epool.tile([128, G], BF16, tag=f"h{dch}_{e%4}")
                        eng(cyc("add")).tensor_tensor(
                            h[:], xlT[dch][:, s * G:(s + 1) * G],
                            xrT[dch][:, d * G:(d + 1) * G],
                            op=mybir.AluOpType.add)
                        leaky_op(cyc("leaky"),
                                 st[dch][:, e * G:(e + 1) * G], h[:])

            def make_fill(pv):
                """PE + small closures for prev block's score/softmax path."""
                fill = []
                st = pv["st"]
                sc_ps = ps_sc.tile([NE, G], F32, tag="sc")

                def mk_sc(e, dch):
                    def f():
                        blk = (dch * NE + e) * NE
                        nc.tensor.matmul(
                            sc_ps[:], attbl_sb[:, blk:blk + NE],
                            st[dch][:, e * G:(e + 1) * G],
                            start=(e == 0 and dch == 0),
                            stop=(e == NE - 1 and dch == 1))
                    return f

                for e in range(NE):
                    for dch in range(2):
                        fill.append(mk_sc(e, dch))

                ex_sb = spool.tile([NE, G], F32, tag="ex")
                den_sb = spool.tile([NE, G], F32, tag="den")
                alphaT = spool.tile([128, NT * NE], F32, tag="alphaT")
                pv["alphaT"] = alphaT

                def f_exp():
                    nc.scalar.activation(ex_sb[:], sc_ps[:],
                                         mybir.ActivationFunctionType.Exp)
                    if pv["b"] == 0:
                        nc.sync.dma_start(dbg_ex_d[:], ex_sb[:])
                fill.append(f_exp)

                mis1 = ps_sc.tile([128, max(G, 2 * NT * NE)], F32, tag="mis")
                den_ps = mis1[0:NE, 0:G]

                def f_den():
                    nc.tensor.matmul(den_ps, smat_sb[:], ex_sb[:],
                                     start=True, stop=True)
                fill.append(f_den)

                def f_denc():
                    nc.scalar.copy(den_sb[:], den_ps)
                fill.append(f_denc)

                exT_ps = ps_sc.tile([128, max(G, 2 * NT * NE)], F32, tag="mis")
                dT0 = NT * NE

                def f_tr():
                    for t in range(NT):
                        nc.tensor.transpose(
                            exT_ps[:, t * NE:(t + 1) * NE],
                            ex_sb[:, t * 128:(t + 1) * 128],
                            identf_sb[0:NE, 0:NE])
                        nc.tensor.transpose(
                            exT_ps[:, dT0 + t * NE:dT0 + (t + 1) * NE],
                            den_sb[:, t * 128:(t + 1) * 128],
                            identf_sb[0:NE, 0:NE])
                fill.append(f_tr)

                def f_alpha():
                    rdenT = spool.tile([128, NT * NE], F32, tag="rdenT")
                    nc.vector.reciprocal(rdenT[:],
                                         exT_ps[:, dT0:dT0 + dT0])
                    nc.vector.tensor_tensor(
                        alphaT[:], exT_ps[:, 0:dT0], rdenT[:],
                        op=mybir.AluOpType.mult)
                    if pv["b"] == 0:
                        nc.sync.dma_start(dbg_al_d[:], alphaT[:])
                fill.append(f_alpha)
                return fill

            def emit_agg(pv):
                """Aggregation + output DMA for prev block (late in queues)."""
                alphaT = pv["alphaT"]
                xl_gms = pv["xl_gms"]
                bb = pv["b"]
                for t in range(NT):
                    xl_gm = xl_gms[t]
                    out_t = opool.tile([128, NN * C], BF16, tag="out_t")
                    for d in range(NN):
                        es = IN_EDGES[d]
                        e0 = es[0]
                        ie = cyc("aggi")
                        if ie == "scalar":
                            nc.scalar.activation(
                                out_t[:, d * C:(d + 1) * C],
                                xl_gm[:, d * C:(d + 1) * C],
                                mybir.ActivationFunctionType.Copy,
                                scale=alphaT[:, t * NE + e0:t * NE + e0 + 1])
                        else:
                            eng(ie).tensor_scalar_mul(
                                out_t[:, d * C:(d + 1) * C],
                                xl_gm[:, d * C:(d + 1) * C],
                                alphaT[:, t * NE + e0:t * NE + e0 + 1])
                        for e in es[1:]:
                            s = EDGES[e][0]
                            eng(cyc("agg")).scalar_tensor_tensor(
                                out_t[:, d * C:(d + 1) * C],
                                xl_gm[:, s * C:(s + 1) * C],
                                alphaT[:, t * NE + e:t * NE + e + 1],
                                out_t[:, d * C:(d + 1) * C],
                                op0=mybir.AluOpType.mult,
                                op1=mybir.AluOpType.add)
                    nc.sync.dma_start(
                        out_d[bb * G + t * 128:bb * G + (t + 1) * 128, :],
                        out_t[:])

            prev = None       # state dict of block b-1
            for b in range(NBLK + 1):
                cur = None
                fill = make_fill(prev) if prev is not None else []
                fi = 0

                def drain(k):
                    nonlocal fi
                    for _ in range(k):
                        if fi < len(fill):
                            fill[fi]()
                            fi += 1

                if b < NBLK:
                    cur = {"b": b}
                    # ---- load xT block ----
                    xt = []
                    for chk in range(2):
                        t_ = xpool.tile([128, NGT], BF16, tag=f"xt{chk}")
                        nc.sync.dma_start(
                            t_[:], xT_d[chk * 128:(chk + 1) * 128,
                                        b * NGT:(b + 1) * NGT])
                        xt.append(t_)

                    # ---- channel-major projections ----
                    # xlT/xrT: [128, 9, G] per (w-matrix, out-chunk)
                    xlT = [prpool.tile([128, NN * G], BF16, tag=f"xlT{d}",
                                       name=f"xlT{d}") for d in range(2)]
                    xrT = [prpool.tile([128, NN * G], BF16, tag=f"xrT{d}",
                                       name=f"xrT{d}") for d in range(2)]
                    cur["xlT"], cur["xrT"] = xlT, xrT
                    cur["st"] = [epool.tile([128, NE * G], FP8,
                                            tag=f"st{d}", name=f"st{d}")
                                 for d in range(2)]
                    ui = 0
                    for (wsb, dest) in ((wl_sb, xlT), (wr_sb, xrT)):
                        for dch in range(2):
                            s = 0
                            while s < NN:
                                npair = min(2, NN - s)
                                ps = ps_proj.tile([128, 2 * G], F32,
                                                  tag="ps_proj")
                                for j in range(npair):
                                    for kc in range(2):
                                        nc.tensor.matmul(
                                            ps[:, j * G:(j + 1) * G],
                                            wsb[:, kc * C + dch * 128:
                                                kc * C + dch * 128 + 128],
                                            xt[kc][:, (s + j) * G:
                                                   (s + j + 1) * G],
                                            start=(kc == 0), stop=(kc == 1))
                                copy_op(cyc("pcopy"),
                                        dest[dch][:, s * G:(s + npair) * G],
                                        ps[:, 0:npair * G])
                                drain(cfg.fill_proj[ui]
                                      if ui < len(cfg.fill_proj) else 2)
                                ui += 1
                                s += npair

                    # this block's edge ops (consumed by next iteration)
                    emit_edge_ops(cur)
                    if b == 0:
                        dbg_st = spool.tile([128, 4 * G], F32, tag="dbgst")
                        nc.scalar.copy(dbg_st[:], cur["st"][0][:, 0:4 * G])
                        nc.sync.dma_start(dbg_st_d[0:128, 0:4, :],
                                          dbg_st[:])

                    # ---- graph-major xl projection ----
                    xl_gms = [gmpool.tile([128, NN * C], BF16,
                                          tag=f"xl_gm{t}", name=f"xl_gm{t}")
                              for t in range(NT)]
                    cur["xl_gms"] = xl_gms
                    for t in range(NT):
                        s = 0
                        while s < NN:
                            npair = min(2, NN - s)
                            ps = ps_gm.tile([128, 512], F32, tag="ps_gm")
                            for j in range(npair):
                                for kc in range(2):
                                    nc.tensor.matmul(
                                        ps[:, j * C:(j + 1) * C],
                                        xt[kc][:, (s + j) * G + t * 128:
                                               (s + j) * G + (t + 1) * 128],
                                        wl_sb[:, kc * C:(kc + 1) * C],
                                        start=(kc == 0), stop=(kc == 1))
                            copy_op(cyc("gcopy"),
                                    xl_gms[t][:, s * C:(s + npair) * C],
                                    ps[:, 0:npair * C])
                            drain(cfg.fill_gm)
                            s += npair

                drain(len(fill))      # flush remaining prev closures
                if prev is not None:
                    emit_agg(prev)
                prev = cur

    nc.compile()
    return nc


def make_host_inputs(x, W_l, W_r, att, cfg: Cfg):
    """Builds the per-core input maps (host-side sharding + layout prep)."""
    x = np.asarray(x, dtype=np.float32)
    W_l = np.ascontiguousarray(np.asarray(W_l, dtype=np.float32))
    W_r = np.ascontiguousarray(np.asarray(W_r, dtype=np.float32))
    att = np.asarray(att, dtype=np.float32)
    bf = ml_dtypes.bfloat16
    f8 = ml_dtypes.bfloat16  # bisect

    # att (x) onehot(e) stationary blocks for the leaky-relu'd score dot
    attbl = np.zeros((128, 2, NE, NE), dtype=np.float32)
    for dch in range(2):
        for e in range(NE):
            attbl[:, dch, e, e] = att[dch * 128:(dch + 1) * 128]
    attbl = attbl.reshape(128, 2 * NE * NE).astype(f8)

    smat = np.zeros((NE, NE), dtype=np.float32)
    for e1, (_s1, d1) in enumerate(EDGES):
        for e2, (_s2, d2) in enumerate(EDGES):
            if d1 == d2:
                smat[e1, e2] = 1.0

    ident = np.eye(128, dtype=np.float32)

    in_maps = []
    for c in range(N_CORES):
        xc = x[c * BC:(c + 1) * BC]                       # [BC, 9, 256]
        xT = np.ascontiguousarray(
            xc.reshape(NBLK, G, NN, C).transpose(3, 0, 2, 1).reshape(
                C, NBLK * NGT).astype(bf))
        in_maps.append({
            "xT": xT,
            "wl": W_l.astype(bf),
            "wr": W_r.astype(bf),
            "attbl": attbl,
            "smat": smat,
            "identf": ident,
        })
    return in_maps


_CACHE = {}


def _cfg_key(cfg: Cfg):
    return (cfg.add_engines, cfg.leaky_engines, cfg.agg_engines,
            cfg.aggi_engines, cfg.pcopy_engines, cfg.gcopy_engines,
            cfg.fill_proj, cfg.fill_gm)


def _get_program(cfg: Cfg):
    key = _cfg_key(cfg)
    if key not in _CACHE:
        _CACHE[key] = build_program(cfg)
    return _CACHE[key]


def kernel(x, W_l, W_r, att, bias, cfg: Cfg = None, trace: bool = False,
           _results_holder: dict = None, **run_kwargs):
    cfg = cfg or Cfg()
    nc = _get_program(cfg)
    in_maps = make_host_inputs(x, W_l, W_r, att, cfg)
    res = run_bass_kernel_spmd(nc, in_maps, core_ids=list(range(N_CORES)),
                               trace=trace, **run_kwargs)
    if _results_holder is not None:
        _results_holder["res"] = res
    outs = [np.asarray(r["out"], dtype=np.float32).reshape(BC, NN, C)
            for r in res.results]
    out = np.concatenate(outs, axis=0)
    bias = np.asarray(bias, dtype=np.float32)
    if np.any(bias):
        out = out + bias
    return out.astype(np.float32)
